# revision 7
# baseline (speedup 1.0000x reference)
"""DiffGraphormer TransformerConv edge-classifier kernel for 8x Trainium2 NeuronCores.

Edge-parallel, dst-range-sharded, all-matmul design:
  - Core c owns dst nodes [c*NPC, (c+1)*NPC). Edges are routed to the core
    owning their dst, sorted by dst, and padded into fixed 128-node windows so
    segment-softmax/segment-sum become PE one-hot matmuls accumulating in PSUM
    (SPMD-uniform program; only per-core input data differs).
  - Host ships per-edge transposed node features x_t[dst]/x_t[src] (bf16), so
    q/k/v/enc are per-edge matmuls (no hardware gather engine needed).
  - Stage B scatters [msg*exp(alpha) | exp(alpha)] into a per-node u-table via
    one-hot matmuls. Stage C normalizes (+bias folds) and computes
    yA = agg @ Wc per own node; yA is AllGathered.
  - Stages D1 (dst-sorted) / D2 (src-sorted) select yA[dst] / yA[src] per edge
    via one-hot matmuls. Host reassembles:
      out = yA[dst] + yA[src] + (x[dst]+x[src]) @ (Wn@Ws@Wc) + const.
"""

import sys

for _p in ("/opt/trn_rl_repo",):
    if _p not in sys.path:
        sys.path.insert(0, _p)

import numpy as np

import concourse.bass as bass
import concourse.tile as tile
from concourse import bacc, mybir

N, E = 50000, 1000000
IN_FEATS, HID, HEADS, EDGE_FEAT, NUM_CLASSES = 128, 128, 8, 64, 2
CPH = HID // HEADS  # 16 channels per head
NCORES = 8
NPC = N // NCORES   # 6250 nodes per core
WIN = 128           # nodes per window
EPS = 1e-16

F32 = mybir.dt.float32
BF16 = mybir.dt.bfloat16
FP16 = mybir.dt.float16
I8 = mybir.dt.int8


def _round_up(a, b):
    return (a + b - 1) // b * b


# ----------------------------------------------------------------------------
# Host-side preparation
# ----------------------------------------------------------------------------

def _build_slots(order_ids, win_cnt, n_win, s_sub):
    """slot -> original edge id map. Window w owns slots [w*s_sub*128, ...);
    its (contiguous) edges fill the head, the rest are -1 pads."""
    slots = np.full(n_win * s_sub * 128, -1, dtype=np.int64)
    pos = 0
    base = 0
    for w in range(n_win):
        n_w = int(win_cnt[w])
        slots[base:base + n_w] = order_ids[pos:pos + n_w]
        pos += n_w
        base += s_sub * 128
    return slots


def prep_host(inputs):
    x_t = np.asarray(inputs["x_t"], np.float32)
    edge_attr_t = np.asarray(inputs["edge_attr_t"], np.float32)
    edge_attr_t_dt = np.asarray(inputs["edge_attr_t_dt"], np.float32)
    edge_index = np.asarray(inputs["edge_index"], np.int64)
    src, dst = edge_index[0], edge_index[1]

    g = lambda k: np.asarray(inputs[k], np.float32)
    Wn, bn, We, be = g("Wn"), g("bn"), g("We"), g("be")
    Wq, bq, Wk, bk = g("Wq"), g("bq"), g("Wk"), g("bk")
    Wv, bv, Ws, bs = g("Wv"), g("bv"), g("Ws"), g("bs")
    Wc, bc = g("Wc"), g("bc")

    # fold node encoder into every projection (x is never materialized)
    Wq2 = Wn @ Wq; bq2 = bn @ Wq + bq
    Wk2 = Wn @ Wk; bk2 = bn @ Wk + bk
    Wv2 = Wn @ Wv; bv2 = bn @ Wv + bv
    Ws2 = Wn @ Ws; bs2 = bn @ Ws + bs
    Wsc = Ws2 @ Wc                            # [128, 2] x-part of y
    Wpm = np.concatenate([We, -We], axis=0)   # [128, 128]: enc from [dt; t]
    U1 = np.zeros((IN_FEATS, HEADS), np.float32)
    U2 = np.zeros((IN_FEATS, HEADS), np.float32)
    for h in range(HEADS):
        sl = slice(h * CPH, (h + 1) * CPH)
        U1[:, h] = Wq2[:, sl] @ bk2[sl]
        U2[:, h] = Wk2[:, sl] @ bq2[sl]
    cv = bv2 + be                             # [128] constant message part
    out_const = 2.0 * (bs2 @ Wc) + bc         # [2] added on host
    has_bias = bool(np.abs(U1).max() > 0 or np.abs(U2).max() > 0
                    or np.abs(cv).max() > 0)

    n_win1 = (NPC + WIN - 1) // WIN           # local windows (49)
    n_win2 = (N + WIN - 1) // WIN             # global windows (391)

    raw = []
    s_sub_max, s2_max = 1, 1
    for c in range(NCORES):
        e_ids = np.nonzero((dst // NPC) == c)[0]
        d_rel = dst[e_ids] - c * NPC
        o1 = np.argsort(d_rel, kind="stable")
        e1 = e_ids[o1]
        w1 = np.bincount(d_rel[o1] // WIN, minlength=n_win1)
        if len(e1):
            s_sub_max = max(s_sub_max, int(np.ceil(w1.max() / 128)))
        o2 = np.argsort(src[e_ids], kind="stable")
        e2 = e_ids[o2]
        w2 = np.bincount(src[e2] // WIN, minlength=n_win2)
        if len(e2):
            s2_max = max(s2_max, int(np.ceil(w2.max() / 128)))
        raw.append((e1, w1, e2, w2))

    s_sub = _round_up(s_sub_max, 4)
    s2 = _round_up(s2_max, 4)
    nsub1 = n_win1 * s_sub
    nsub2 = n_win2 * s2

    per_core = []
    for c in range(NCORES):
        e1, w1, e2, w2 = raw[c]
        slots1 = _build_slots(e1, w1, n_win1, s_sub)
        slots2 = _build_slots(e2, w2, n_win2, s2)
        v1 = slots1 >= 0
        v2 = slots2 >= 0
        se1 = np.where(v1, slots1, 0)
        se2 = np.where(v2, slots2, 0)

        xd = np.where(v1[:, None], x_t[dst[se1]], 0.0).astype(np.float32)
        xs = np.where(v1[:, None], x_t[src[se1]], 0.0).astype(np.float32)
        at = np.where(v1[:, None],
                      np.concatenate([edge_attr_t_dt[se1], edge_attr_t[se1]],
                                     axis=1), 0.0).astype(np.float32)
        # subtile-major feature-transposed streams [nsub, 128feat, 128edge]
        xdT = np.ascontiguousarray(xd.reshape(nsub1, 128, 128).transpose(0, 2, 1))
        xsT = np.ascontiguousarray(xs.reshape(nsub1, 128, 128).transpose(0, 2, 1))
        atT = np.ascontiguousarray(at.reshape(nsub1, 128, 128).transpose(0, 2, 1))

        d_in_win = np.where(v1, (dst[se1] - c * NPC) % WIN, 0)
        dstcol = d_in_win.reshape(nsub1, 128, 1).astype(np.float32)
        dstrow = d_in_win.reshape(nsub1, 128).astype(np.int8)
        s_in_win = np.where(v2, src[se2] % WIN, 0)
        srcrow = s_in_win.reshape(nsub2, 128).astype(np.int8)

        pc = np.zeros(n_win1 * WIN, np.float32)
        pc[np.arange(n_win1) * WIN] = s_sub * 128 - w1
        pc = pc.reshape(n_win1, 128, 1)

        per_core.append(dict(
            slots1=slots1, valid1=v1, slots2=slots2, valid2=v2,
            xdT=xdT, xsT=xsT, atT=atT, dstcol=dstcol,
            dstrow=dstrow.reshape(-1), srcrow=srcrow.reshape(-1), pc=pc,
        ))

    consts = dict(Wq2=Wq2, Wk2=Wk2, Wv2=Wv2, Wpm=Wpm, Wsc=Wsc, Wc=Wc,
                  U1=U1, U2=U2, cv=cv, out_const=out_const, has_bias=has_bias)
    meta = dict(s_sub=s_sub, s2=s2, nsub1=nsub1, nsub2=nsub2,
                n_win1=n_win1, n_win2=n_win2)
    return per_core, consts, meta


# ----------------------------------------------------------------------------
# Device program (SPMD, identical on all 8 cores)
# ----------------------------------------------------------------------------

def build_program(meta, has_bias):
    s_sub, s2 = meta["s_sub"], meta["s2"]
    nsub1, nsub2 = meta["nsub1"], meta["nsub2"]
    n_win1, n_win2 = meta["n_win1"], meta["n_win2"]
    gw1 = s_sub // 4
    gw2 = s2 // 4
    n_nodes1 = n_win1 * WIN
    n_nodes2 = n_win2 * WIN

    nc = bacc.Bacc("TRN2", target_bir_lowering=False, debug=True)

    xdT_d = nc.dram_tensor("xdT", [nsub1, 128, 128], BF16, kind="ExternalInput")
    xsT_d = nc.dram_tensor("xsT", [nsub1, 128, 128], BF16, kind="ExternalInput")
    atT_d = nc.dram_tensor("atT", [nsub1, 128, 128], BF16, kind="ExternalInput")
    dstcol_d = nc.dram_tensor("dstcol", [nsub1, 128, 1], BF16, kind="ExternalInput")
    dstrow_d = nc.dram_tensor("dstrow", [nsub1 * 128], I8, kind="ExternalInput")
    srcrow_d = nc.dram_tensor("srcrow", [nsub2 * 128], I8, kind="ExternalInput")
    pc_d = nc.dram_tensor("pc", [n_win1, 128, 1], F32, kind="ExternalInput")
    wq_d = nc.dram_tensor("wq", [128, 128], BF16, kind="ExternalInput")
    wk_d = nc.dram_tensor("wk", [128, 128], BF16, kind="ExternalInput")
    wv_d = nc.dram_tensor("wv", [128, 128], BF16, kind="ExternalInput")
    wpm_d = nc.dram_tensor("wpm", [128, 128], BF16, kind="ExternalInput")
    wsc_d = nc.dram_tensor("wsc", [128, 2], BF16, kind="ExternalInput")
    wc_d = nc.dram_tensor("wc", [128, 2], F32, kind="ExternalInput")
    m2_d = nc.dram_tensor("m2", [128, 128], BF16, kind="ExternalInput")
    m1_d = nc.dram_tensor("m1", [128, 1], I8, kind="ExternalInput")
    cv_d = nc.dram_tensor("cvrow", [1, 128], F32, kind="ExternalInput")
    u1_d = nc.dram_tensor("u1", [128, 8], BF16, kind="ExternalInput")
    u2_d = nc.dram_tensor("u2", [128, 8], BF16, kind="ExternalInput")

    out1_d = nc.dram_tensor("out1", [nsub1, 128, 2], F32, kind="ExternalOutput")
    out2_d = nc.dram_tensor("out2", [nsub2, 128, 2], F32, kind="ExternalOutput")

    u_d = nc.dram_tensor("u_tab", [n_nodes1, 136], F32)
    xy_d = nc.dram_tensor("xy_tab", [nsub1, 128, 2], F32)
    ya_own_d = nc.dram_tensor("ya_own", [n_nodes1, 2], F32)
    ya_all_d = nc.dram_tensor("ya_all", [n_nodes2, 2], F32, addr_space="Shared")

    with tile.TileContext(nc) as tc:
        with tc.tile_pool(name="static", bufs=1) as stat:
            def load_const(name_d, shape, dt):
                t = stat.tile(shape, dt, tag="const_" + name_d.name)
                nc.sync.dma_start(out=t[:], in_=name_d[:])
                return t
            wq_s = load_const(wq_d, [128, 128], BF16)
            wk_s = load_const(wk_d, [128, 128], BF16)
            wv_s = load_const(wv_d, [128, 128], BF16)
            wpm_s = load_const(wpm_d, [128, 128], BF16)
            wsc_s = load_const(wsc_d, [128, 2], BF16)
            wc_s = load_const(wc_d, [128, 2], F32)
            m2_s = load_const(m2_d, [128, 128], BF16)
            m1_s = load_const(m1_d, [128, 1], I8)
            cv_s = stat.tile([128, 128], F32)
            nc.sync.dma_start(out=cv_s[:], in_=bass.AP(
                tensor=cv_d[:].tensor, offset=0, ap=[[0, 128], [1, 128]]))
            if has_bias:
                u1_s = load_const(u1_d, [128, 8], BF16)
                u2_s = load_const(u2_d, [128, 8], BF16)

            # ================= B: attention + scatter =================
            with tc.tile_pool(name="bsb", bufs=3) as bsb, \
                 tc.tile_pool(name="bps", bufs=2, space="PSUM") as bps, \
                 tc.tile_pool(name="bps2", bufs=1, space="PSUM") as bps2, \
                 tc.tile_pool(name="ups", bufs=1, space="PSUM") as ups:
                for w in range(n_win1):
                    u_ps = ups.tile([128, 136], F32, tag="u")
                    for gi in range(gw1):
                        grp = w * gw1 + gi
                        sl = slice(grp * 4, (grp + 1) * 4)
                        xd_t = bsb.tile([128, 4, 128], BF16, tag="xd")
                        xs_t = bsb.tile([128, 4, 128], BF16, tag="xs")
                        at_t = bsb.tile([128, 4, 128], BF16, tag="at")
                        dc_t = bsb.tile([128, 4], BF16, tag="dc")
                        nc.sync.dma_start(out=xd_t[:], in_=xdT_d[sl].rearrange("s f e -> f s e"))
                        nc.sync.dma_start(out=xs_t[:], in_=xsT_d[sl].rearrange("s f e -> f s e"))
                        nc.sync.dma_start(out=at_t[:], in_=atT_d[sl].rearrange("s f e -> f s e"))
                        nc.sync.dma_start(out=dc_t[:], in_=dstcol_d[sl].rearrange("s e one -> e (s one)"))

                        qd_ps = bps.tile([128, 4, 128], F32, tag="qd")
                        k_ps = bps.tile([128, 4, 128], F32, tag="k")
                        v_ps = bps.tile([128, 4, 128], F32, tag="v")
                        sm_ps = bps2.tile([128, 4, 10], F32, tag="sm")
                        xy_ps = sm_ps[:, :, 0:2]
                        if has_bias:
                            be_ps = sm_ps[:, :, 2:10]
                        for s in range(4):
                            nc.tensor.matmul(out=qd_ps[:, s, :], lhsT=xd_t[:, s, :], rhs=wq_s[:],
                                             start=True, stop=True, skip_group_check=True)
                            nc.tensor.matmul(out=k_ps[:, s, :], lhsT=xs_t[:, s, :], rhs=wk_s[:],
                                             start=True, stop=True, skip_group_check=True)
                            nc.tensor.matmul(out=v_ps[:, s, :], lhsT=xs_t[:, s, :], rhs=wv_s[:],
                                             start=True, stop=False, skip_group_check=True)
                            nc.tensor.matmul(out=v_ps[:, s, :], lhsT=at_t[:, s, :], rhs=wpm_s[:],
                                             start=False, stop=True, skip_group_check=True)
                            nc.tensor.matmul(out=xy_ps[:, s, :], lhsT=xd_t[:, s, :], rhs=wsc_s[:],
                                             start=True, stop=False, skip_group_check=True)
                            nc.tensor.matmul(out=xy_ps[:, s, :], lhsT=xs_t[:, s, :], rhs=wsc_s[:],
                                             start=False, stop=True, skip_group_check=True)
                            if has_bias:
                                nc.tensor.matmul(out=be_ps[:, s, :], lhsT=xd_t[:, s, :], rhs=u1_s[:],
                                                 start=True, stop=False, skip_group_check=True)
                                nc.tensor.matmul(out=be_ps[:, s, :], lhsT=xs_t[:, s, :], rhs=u2_s[:],
                                                 start=False, stop=True, skip_group_check=True)

                        qd_sb = bsb.tile([128, 4, 128], BF16, tag="qdsb")
                        nc.scalar.copy(out=qd_sb[:], in_=qd_ps[:])
                        k_sb = bsb.tile([128, 4, 128], BF16, tag="ksb")
                        nc.scalar.copy(out=k_sb[:], in_=k_ps[:])
                        prod = bsb.tile([128, 4, 128], BF16, tag="prod")
                        nc.vector.tensor_mul(prod[:], qd_sb[:], k_sb[:])
                        alpha = bsb.tile([128, 4, 8], F32, tag="alpha")
                        nc.vector.tensor_reduce(
                            out=alpha[:],
                            in_=prod[:].rearrange("e s (h c) -> e s h c", h=HEADS),
                            axis=mybir.AxisListType.X, op=mybir.AluOpType.add)
                        if has_bias:
                            nc.vector.tensor_add(alpha[:], alpha[:], be_ps)
                        payload = bsb.tile([128, 4, 136], BF16, tag="pay")
                        nc.scalar.activation(
                            out=payload[:, :, 128:136], in_=alpha[:],
                            func=mybir.ActivationFunctionType.Exp, scale=0.25)
                        nc.vector.tensor_mul(
                            payload[:, :, 0:128].rearrange("e s (h c) -> e s h c", h=HEADS),
                            v_ps[:].rearrange("e s (h c) -> e s h c", h=HEADS),
                            payload[:, :, 128:136].unsqueeze(3).to_broadcast([128, 4, 8, CPH]))
                        selT = bsb.tile([128, 4, 128], BF16, tag="selT")
                        nc.vector.tensor_tensor(
                            out=selT[:],
                            in0=dc_t[:].unsqueeze(2).to_broadcast([128, 4, 128]),
                            in1=m2_s[:].unsqueeze(1).to_broadcast([128, 4, 128]),
                            op=mybir.AluOpType.is_equal)
                        for s in range(4):
                            nc.tensor.matmul(
                                out=u_ps[:], lhsT=selT[:, s, :], rhs=payload[:, s, :],
                                start=(gi == 0 and s == 0),
                                stop=(gi == gw1 - 1 and s == 3),
                                skip_group_check=True)
                        xy_sb = bsb.tile([128, 4, 2], F32, tag="xysb")
                        nc.scalar.copy(out=xy_sb[:], in_=xy_ps)
                        nc.sync.dma_start(out=xy_d[sl].rearrange("s e t -> e s t"), in_=xy_sb[:])
                    u_sb = bsb.tile([128, 136], F32, tag="usb")
                    nc.scalar.copy(out=u_sb[:], in_=u_ps[:])
                    nc.sync.dma_start(out=u_d[w * WIN:(w + 1) * WIN, :], in_=u_sb[:])

            # ================= C: normalize + yA =================
            from concourse.masks import make_identity
            with tc.tile_pool(name="csb", bufs=3) as csb, \
                 tc.tile_pool(name="cps", bufs=2, space="PSUM") as cps:
                ident = csb.tile([128, 128], F32, tag="ident")
                make_identity(nc, ident[:])
                for w in range(n_win1):
                    u_sb = csb.tile([128, 136], F32, tag="cu")
                    nc.sync.dma_start(out=u_sb[:], in_=u_d[w * WIN:(w + 1) * WIN, :])
                    pc_t = csb.tile([128, 1], F32, tag="cpc")
                    nc.sync.dma_start(out=pc_t[:], in_=pc_d[w])
                    dcr = csb.tile([128, 8], F32, tag="cdc")
                    nc.vector.tensor_sub(dcr[:], u_sb[:, 128:136],
                                         pc_t[:].to_broadcast([128, 8]))
                    den = csb.tile([128, 8], F32, tag="cden")
                    nc.vector.tensor_scalar_add(den[:], dcr[:], EPS)
                    rcp = csb.tile([128, 8], F32, tag="crcp")
                    nc.vector.reciprocal(out=rcp[:], in_=den[:])
                    agg = csb.tile([128, 128], F32, tag="cagg")
                    if has_bias:
                        nc.vector.tensor_mul(
                            agg[:].rearrange("n (h c) -> n h c", h=HEADS),
                            cv_s[:].rearrange("n (h c) -> n h c", h=HEADS),
                            dcr[:].unsqueeze(2).to_broadcast([128, 8, CPH]))
                        nc.vector.tensor_add(agg[:], agg[:], u_sb[:, 0:128])
                    else:
                        nc.vector.tensor_copy(out=agg[:], in_=u_sb[:, 0:128])
                    nc.vector.tensor_mul(
                        agg[:].rearrange("n (h c) -> n h c", h=HEADS),
                        agg[:].rearrange("n (h c) -> n h c", h=HEADS),
                        rcp[:].unsqueeze(2).to_broadcast([128, 8, CPH]))
                    aggT_ps = cps.tile([128, 128], F32, tag="caggT")
                    nc.tensor.transpose(out=aggT_ps[:], in_=agg[:], identity=ident[:])
                    aggT = csb.tile([128, 128], F32, tag="caggTs")
                    nc.scalar.copy(out=aggT[:], in_=aggT_ps[:])
                    ya_ps = cps.tile([128, 2], F32, tag="cya")
                    nc.tensor.matmul(out=ya_ps[:], lhsT=aggT[:], rhs=wc_s[:],
                                     start=True, stop=True, skip_group_check=True)
                    ya_sb = csb.tile([128, 2], F32, tag="cyasb")
                    nc.scalar.copy(out=ya_sb[:], in_=ya_ps[:])
                    nc.sync.dma_start(out=ya_own_d[w * WIN:(w + 1) * WIN, :], in_=ya_sb[:])

                nc.gpsimd.collective_compute(
                    "AllGather", mybir.AluOpType.bypass,
                    replica_groups=[list(range(NCORES))],
                    ins=[ya_own_d[0:NPC, :]], outs=[ya_all_d[0:NCORES * NPC, :]])
                if n_nodes2 > NCORES * NPC:
                    zt = csb.tile([128, 2], F32, tag="czt")
                    nc.vector.memset(zt[:], 0.0)
                    pad = n_nodes2 - NCORES * NPC
                    nc.sync.dma_start(out=ya_all_d[NCORES * NPC:n_nodes2, :],
                                      in_=zt[0:pad, :])

            # ================= D: per-edge y selection =================
            def d_pass(n_win, g_per_w, row_d, ytab_d, out_d, add_xy, pool_pfx):
                with tc.tile_pool(name=pool_pfx + "sb", bufs=3) as dsb, \
                     tc.tile_pool(name=pool_pfx + "ps", bufs=2, space="PSUM") as dps:
                    for w in range(n_win):
                        yw = dsb.tile([128, 2], F32, tag="dyw")
                        nc.sync.dma_start(out=yw[:], in_=ytab_d[w * WIN:(w + 1) * WIN, :])
                        yw16 = dsb.tile([128, 2], FP16, tag="dyw16")
                        nc.scalar.copy(out=yw16[:], in_=yw[:])
                        for gi in range(g_per_w):
                            grp = w * g_per_w + gi
                            sl = slice(grp * 4, (grp + 1) * 4)
                            rows = dsb.tile([128, 4, 128], I8, tag="drow")
                            nc.sync.dma_start(
                                out=rows[:],
                                in_=bass.AP(tensor=row_d[:].tensor, offset=grp * 512,
                                            ap=[[0, 128], [128, 4], [1, 128]]))
                            sel = dsb.tile([128, 4, 128], FP16, tag="dsel")
                            nc.vector.tensor_tensor(
                                out=sel[:],
                                in0=m1_s[:].unsqueeze(2).to_broadcast([128, 4, 128]),
                                in1=rows[:], op=mybir.AluOpType.is_equal)
                            smm = dps.tile([128, 4, 2], F32, tag="dsmm")
                            for s in range(4):
                                nc.tensor.matmul(out=smm[:, s, :], lhsT=sel[:, s, :],
                                                 rhs=yw16[:], start=True, stop=True,
                                                 skip_group_check=True)
                            o_sb = dsb.tile([128, 4, 2], F32, tag="dosb")
                            if add_xy:
                                xy_t = dsb.tile([128, 4, 2], F32, tag="dxy")
                                nc.sync.dma_start(out=xy_t[:], in_=xy_d[sl].rearrange("s e t -> e s t"))
                                nc.vector.tensor_add(o_sb[:], smm[:], xy_t[:])
                            else:
                                nc.vector.tensor_copy(out=o_sb[:], in_=smm[:])
                            nc.sync.dma_start(out=out_d[sl].rearrange("s e t -> e s t"), in_=o_sb[:])

            d_pass(n_win1, gw1, dstrow_d, ya_own_d, out1_d, True, "d1")
            d_pass(n_win2, gw2, srcrow_d, ya_all_d, out2_d, False, "d2")

    nc.compile()
    return nc


_CACHE = {}


def _get_compiled(meta, has_bias):
    key = (meta["s_sub"], meta["s2"], meta["nsub1"], meta["nsub2"],
           meta["n_win1"], meta["n_win2"], has_bias)
    if key not in _CACHE:
        _CACHE[key] = build_program(meta, has_bias)
    return _CACHE[key]


def make_in_maps(per_core, consts):
    bfnp = mybir.dt.np(BF16)
    bf = lambda a: np.ascontiguousarray(a).astype(bfnp)
    m2 = np.tile(np.arange(128, dtype=np.float32)[None, :], (128, 1))
    m1 = np.arange(128, dtype=np.int8)[:, None]
    in_maps = []
    for pcd in per_core:
        in_maps.append({
            "xdT": bf(pcd["xdT"]), "xsT": bf(pcd["xsT"]), "atT": bf(pcd["atT"]),
            "dstcol": bf(pcd["dstcol"]), "dstrow": pcd["dstrow"],
            "srcrow": pcd["srcrow"], "pc": pcd["pc"],
            "wq": bf(consts["Wq2"]), "wk": bf(consts["Wk2"]),
            "wv": bf(consts["Wv2"]), "wpm": bf(consts["Wpm"]),
            "wsc": bf(consts["Wsc"]), "wc": np.ascontiguousarray(consts["Wc"]),
            "m2": bf(m2), "m1": m1,
            "cvrow": np.ascontiguousarray(consts["cv"][None, :]),
            "u1": bf(consts["U1"]), "u2": bf(consts["U2"]),
        })
    return in_maps


def assemble_output(res_list, per_core, consts, meta):
    out = np.zeros((E, NUM_CLASSES), np.float32)
    for c in range(NCORES):
        pcd = per_core[c]
        r = res_list[c]
        o1 = np.asarray(r["out1"], np.float32).reshape(-1, NUM_CLASSES)
        o2 = np.asarray(r["out2"], np.float32).reshape(-1, NUM_CLASSES)
        out[pcd["slots1"][pcd["valid1"]]] += o1[pcd["valid1"]]
        out[pcd["slots2"][pcd["valid2"]]] += o2[pcd["valid2"]]
    out += consts["out_const"][None, :]
    return out


def kernel(**inputs):
    from concourse.bass_utils import run_bass_kernel_spmd

    per_core, consts, meta = prep_host(inputs)
    nc = _get_compiled(meta, consts["has_bias"])
    in_maps = make_in_maps(per_core, consts)
    res = run_bass_kernel_spmd(nc, in_maps, list(range(NCORES)))
    return assemble_output(res.results, per_core, consts, meta)


# revision 9
# speedup vs baseline: 1.3300x; 1.3300x over previous
"""DiffGraphormer TransformerConv edge-classifier kernel for 8x Trainium2 NeuronCores.

Edge-parallel, dst-range-sharded, all-matmul design:
  - Core c owns dst nodes [c*NPC, (c+1)*NPC). Edges are routed to the core
    owning their dst, sorted by dst, and padded into fixed 128-node windows so
    segment-softmax/segment-sum become PE one-hot matmuls accumulating in PSUM
    (SPMD-uniform program; only per-core input data differs).
  - Host ships per-edge transposed node features x_t[dst]/x_t[src] (bf16), so
    q/k/v/enc are per-edge matmuls (no hardware gather engine needed).
  - Stage B scatters [msg*exp(alpha) | exp(alpha)] into a per-node u-table via
    one-hot matmuls. Stage C normalizes (+bias folds) and computes
    yA = agg @ Wc per own node; yA is AllGathered.
  - Stages D1 (dst-sorted) / D2 (src-sorted) select yA[dst] / yA[src] per edge
    via one-hot matmuls. Host reassembles:
      out = yA[dst] + yA[src] + (x[dst]+x[src]) @ (Wn@Ws@Wc) + const.
"""

import sys

for _p in ("/opt/trn_rl_repo",):
    if _p not in sys.path:
        sys.path.insert(0, _p)

import numpy as np

import concourse.bass as bass
import concourse.tile as tile
from concourse import bacc, mybir

N, E = 50000, 1000000
IN_FEATS, HID, HEADS, EDGE_FEAT, NUM_CLASSES = 128, 128, 8, 64, 2
CPH = HID // HEADS  # 16 channels per head
NCORES = 8
NPC = N // NCORES   # 6250 nodes per core
WIN = 128           # nodes per window
EPS = 1e-16
PKW = 1540          # packed B-stream row: 512 xd | 512 xs | 512 at | 4 dstcol

F32 = mybir.dt.float32
BF16 = mybir.dt.bfloat16
FP16 = mybir.dt.float16
I8 = mybir.dt.int8


def _round_up(a, b):
    return (a + b - 1) // b * b


# ----------------------------------------------------------------------------
# Host-side preparation
# ----------------------------------------------------------------------------

def _build_slots(order_ids, win_cnt, n_win, s_sub):
    slots = np.full(n_win * s_sub * 128, -1, dtype=np.int64)
    pos = 0
    base = 0
    for w in range(n_win):
        n_w = int(win_cnt[w])
        slots[base:base + n_w] = order_ids[pos:pos + n_w]
        pos += n_w
        base += s_sub * 128
    return slots


def prep_host(inputs):
    x_t = np.asarray(inputs["x_t"], np.float32)
    edge_attr_t = np.asarray(inputs["edge_attr_t"], np.float32)
    edge_attr_t_dt = np.asarray(inputs["edge_attr_t_dt"], np.float32)
    edge_index = np.asarray(inputs["edge_index"], np.int64)
    src, dst = edge_index[0], edge_index[1]

    g = lambda k: np.asarray(inputs[k], np.float32)
    Wn, bn, We, be = g("Wn"), g("bn"), g("We"), g("be")
    Wq, bq, Wk, bk = g("Wq"), g("bq"), g("Wk"), g("bk")
    Wv, bv, Ws, bs = g("Wv"), g("bv"), g("Ws"), g("bs")
    Wc, bc = g("Wc"), g("bc")

    Wq2 = Wn @ Wq; bq2 = bn @ Wq + bq
    Wk2 = Wn @ Wk; bk2 = bn @ Wk + bk
    Wv2 = Wn @ Wv; bv2 = bn @ Wv + bv
    Ws2 = Wn @ Ws; bs2 = bn @ Ws + bs
    Wsc = Ws2 @ Wc
    Wpm = np.concatenate([We, -We], axis=0)
    U1 = np.zeros((IN_FEATS, HEADS), np.float32)
    U2 = np.zeros((IN_FEATS, HEADS), np.float32)
    for h in range(HEADS):
        sl = slice(h * CPH, (h + 1) * CPH)
        U1[:, h] = Wq2[:, sl] @ bk2[sl]
        U2[:, h] = Wk2[:, sl] @ bq2[sl]
    cv = bv2 + be
    out_const = 2.0 * (bs2 @ Wc) + bc
    has_bias = bool(np.abs(U1).max() > 0 or np.abs(U2).max() > 0
                    or np.abs(cv).max() > 0)

    n_win1 = (NPC + WIN - 1) // WIN
    n_win2 = (N + WIN - 1) // WIN

    raw = []
    s_sub_max, s2_max = 1, 1
    for c in range(NCORES):
        e_ids = np.nonzero((dst // NPC) == c)[0]
        d_rel = dst[e_ids] - c * NPC
        o1 = np.argsort(d_rel, kind="stable")
        e1 = e_ids[o1]
        w1 = np.bincount(d_rel[o1] // WIN, minlength=n_win1)
        if len(e1):
            s_sub_max = max(s_sub_max, int(np.ceil(w1.max() / 128)))
        o2 = np.argsort(src[e_ids], kind="stable")
        e2 = e_ids[o2]
        w2 = np.bincount(src[e2] // WIN, minlength=n_win2)
        if len(e2):
            s2_max = max(s2_max, int(np.ceil(w2.max() / 128)))
        raw.append((e1, w1, e2, w2))

    s_sub = _round_up(s_sub_max, 8)   # D1 batches 8 subtiles per window
    s2 = _round_up(s2_max, 4)
    nsub1 = n_win1 * s_sub
    nsub2 = n_win2 * s2
    ngrp1 = nsub1 // 4

    bfnp = mybir.dt.np(BF16)
    per_core = []
    for c in range(NCORES):
        e1, w1, e2, w2 = raw[c]
        slots1 = _build_slots(e1, w1, n_win1, s_sub)
        slots2 = _build_slots(e2, w2, n_win2, s2)
        v1 = slots1 >= 0
        v2 = slots2 >= 0
        se1 = np.where(v1, slots1, 0)
        se2 = np.where(v2, slots2, 0)

        xd = np.where(v1[:, None], x_t[dst[se1]], 0.0).astype(np.float32)
        xs = np.where(v1[:, None], x_t[src[se1]], 0.0).astype(np.float32)
        at = np.where(v1[:, None],
                      np.concatenate([edge_attr_t_dt[se1], edge_attr_t[se1]],
                                     axis=1), 0.0).astype(np.float32)
        d_in_win = np.where(v1, (dst[se1] - c * NPC) % WIN, 0)

        # packed group stream [ngrp1, 128feat, 1540]
        def gview(a):  # [nsub*128, 128] -> [ngrp, 128feat, 4*128edge]
            return (a.reshape(ngrp1, 4, 128, 128).transpose(0, 3, 1, 2)
                     .reshape(ngrp1, 128, 512))
        pk = np.empty((ngrp1, 128, PKW), dtype=bfnp)
        pk[:, :, 0:512] = gview(xd).astype(bfnp)
        pk[:, :, 512:1024] = gview(xs).astype(bfnp)
        pk[:, :, 1024:1536] = gview(at).astype(bfnp)
        dcg = (d_in_win.reshape(ngrp1, 4, 128).transpose(0, 2, 1)
               .astype(np.float32).astype(bfnp))          # [ngrp, 128e, 4s]
        pk[:, :, 1536:1540] = dcg

        dstrow = d_in_win.reshape(-1).astype(np.int8)
        s_in_win = np.where(v2, src[se2] % WIN, 0)
        srcrow = s_in_win.reshape(-1).astype(np.int8)

        pc = np.zeros(n_win1 * WIN, np.float32)
        pc[np.arange(n_win1) * WIN] = s_sub * 128 - w1
        pc = pc.reshape(n_win1, 128, 1)

        per_core.append(dict(
            slots1=slots1, valid1=v1, slots2=slots2, valid2=v2,
            pk=pk, dstrow=dstrow, srcrow=srcrow, pc=pc,
        ))

    consts = dict(Wq2=Wq2, Wk2=Wk2, Wv2=Wv2, Wpm=Wpm, Wsc=Wsc, Wc=Wc,
                  U1=U1, U2=U2, cv=cv, out_const=out_const, has_bias=has_bias)
    meta = dict(s_sub=s_sub, s2=s2, nsub1=nsub1, nsub2=nsub2,
                n_win1=n_win1, n_win2=n_win2)
    return per_core, consts, meta


# ----------------------------------------------------------------------------
# Device program (SPMD, identical on all 8 cores)
# ----------------------------------------------------------------------------

def build_program(meta, has_bias):
    s_sub, s2 = meta["s_sub"], meta["s2"]
    nsub1, nsub2 = meta["nsub1"], meta["nsub2"]
    n_win1, n_win2 = meta["n_win1"], meta["n_win2"]
    gw1 = s_sub // 4
    gw2 = s2 // 4
    ngrp1 = nsub1 // 4
    n_nodes1 = n_win1 * WIN
    n_nodes2 = n_win2 * WIN

    nc = bacc.Bacc("TRN2", target_bir_lowering=False, debug=True)

    pk_d = nc.dram_tensor("pk", [ngrp1, 128, PKW], BF16, kind="ExternalInput")
    dstrow_d = nc.dram_tensor("dstrow", [nsub1 * 128], I8, kind="ExternalInput")
    srcrow_d = nc.dram_tensor("srcrow", [nsub2 * 128], I8, kind="ExternalInput")
    pc_d = nc.dram_tensor("pc", [n_win1, 128, 1], F32, kind="ExternalInput")
    wq_d = nc.dram_tensor("wq", [128, 128], BF16, kind="ExternalInput")
    wk_d = nc.dram_tensor("wk", [128, 128], BF16, kind="ExternalInput")
    wv_d = nc.dram_tensor("wv", [128, 128], BF16, kind="ExternalInput")
    wpm_d = nc.dram_tensor("wpm", [128, 128], BF16, kind="ExternalInput")
    wsc_d = nc.dram_tensor("wsc", [128, 2], BF16, kind="ExternalInput")
    wc_d = nc.dram_tensor("wc", [128, 2], F32, kind="ExternalInput")
    m2_d = nc.dram_tensor("m2", [128, 128], BF16, kind="ExternalInput")
    m1_d = nc.dram_tensor("m1", [128, 1], I8, kind="ExternalInput")
    cv_d = nc.dram_tensor("cvrow", [1, 128], F32, kind="ExternalInput")
    u1_d = nc.dram_tensor("u1", [128, 8], BF16, kind="ExternalInput")
    u2_d = nc.dram_tensor("u2", [128, 8], BF16, kind="ExternalInput")

    out1_d = nc.dram_tensor("out1", [nsub1, 128, 2], F32, kind="ExternalOutput")
    out2_d = nc.dram_tensor("out2", [nsub2, 128, 2], F32, kind="ExternalOutput")

    u_d = nc.dram_tensor("u_tab", [n_nodes1, 136], F32)
    xy_d = nc.dram_tensor("xy_tab", [nsub1, 128, 2], F32)
    ya_own_d = nc.dram_tensor("ya_own", [n_nodes1, 2], F32)
    ya_all_d = nc.dram_tensor("ya_all", [n_nodes2, 2], F32, addr_space="Shared")

    with tile.TileContext(nc) as tc:
        with tc.tile_pool(name="static", bufs=1) as stat:
            def load_const(name_d, shape, dt):
                t = stat.tile(shape, dt, tag="const_" + name_d.name)
                nc.sync.dma_start(out=t[:], in_=name_d[:])
                return t
            wq_s = load_const(wq_d, [128, 128], BF16)
            wk_s = load_const(wk_d, [128, 128], BF16)
            wv_s = load_const(wv_d, [128, 128], BF16)
            wpm_s = load_const(wpm_d, [128, 128], BF16)
            wsc_s = load_const(wsc_d, [128, 2], BF16)
            wc_s = load_const(wc_d, [128, 2], F32)
            m2_s = load_const(m2_d, [128, 128], BF16)
            m1_s = load_const(m1_d, [128, 1], I8)
            cv_s = stat.tile([128, 128], F32)
            nc.sync.dma_start(out=cv_s[:], in_=bass.AP(
                tensor=cv_d[:].tensor, offset=0, ap=[[0, 128], [1, 128]]))
            if has_bias:
                u1_s = load_const(u1_d, [128, 8], BF16)
                u2_s = load_const(u2_d, [128, 8], BF16)

            # ================= B: attention + scatter =================
            with tc.tile_pool(name="bsb", bufs=4) as bsb, \
                 tc.tile_pool(name="bps", bufs=2, space="PSUM") as bps, \
                 tc.tile_pool(name="bps2", bufs=1, space="PSUM") as bps2, \
                 tc.tile_pool(name="ups", bufs=1, space="PSUM") as ups:
                for w in range(n_win1):
                    u_ps = ups.tile([128, 136], F32, tag="u")
                    for gi in range(gw1):
                        grp = w * gw1 + gi
                        sl = slice(grp * 4, (grp + 1) * 4)
                        pk_t = bsb.tile([128, PKW], BF16, tag="pk")
                        nc.sync.dma_start(out=pk_t[:], in_=pk_d[grp])
                        xd_t = pk_t[:, 0:512].rearrange("f (s e) -> f s e", s=4)
                        xs_t = pk_t[:, 512:1024].rearrange("f (s e) -> f s e", s=4)
                        at_t = pk_t[:, 1024:1536].rearrange("f (s e) -> f s e", s=4)
                        dc_t = pk_t[:, 1536:1540]

                        qd_ps = bps.tile([128, 4, 128], F32, tag="qd")
                        k_ps = bps.tile([128, 4, 128], F32, tag="k")
                        v_ps = bps.tile([128, 4, 128], F32, tag="v")
                        sm_ps = bps2.tile([128, 4, 10], F32, tag="sm")
                        xy_ps = sm_ps[:, :, 0:2]
                        if has_bias:
                            be_ps = sm_ps[:, :, 2:10]
                        for s in range(4):
                            nc.tensor.matmul(out=qd_ps[:, s, :], lhsT=xd_t[:, s, :], rhs=wq_s[:],
                                             start=True, stop=True, skip_group_check=True)
                            nc.tensor.matmul(out=k_ps[:, s, :], lhsT=xs_t[:, s, :], rhs=wk_s[:],
                                             start=True, stop=True, skip_group_check=True)
                            nc.tensor.matmul(out=v_ps[:, s, :], lhsT=xs_t[:, s, :], rhs=wv_s[:],
                                             start=True, stop=False, skip_group_check=True)
                            nc.tensor.matmul(out=v_ps[:, s, :], lhsT=at_t[:, s, :], rhs=wpm_s[:],
                                             start=False, stop=True, skip_group_check=True)
                            nc.tensor.matmul(out=xy_ps[:, s, :], lhsT=xd_t[:, s, :], rhs=wsc_s[:],
                                             start=True, stop=False, skip_group_check=True)
                            nc.tensor.matmul(out=xy_ps[:, s, :], lhsT=xs_t[:, s, :], rhs=wsc_s[:],
                                             start=False, stop=True, skip_group_check=True)
                            if has_bias:
                                nc.tensor.matmul(out=be_ps[:, s, :], lhsT=xd_t[:, s, :], rhs=u1_s[:],
                                                 start=True, stop=False, skip_group_check=True)
                                nc.tensor.matmul(out=be_ps[:, s, :], lhsT=xs_t[:, s, :], rhs=u2_s[:],
                                                 start=False, stop=True, skip_group_check=True)

                        qd_sb = bsb.tile([128, 4, 128], BF16, tag="qdsb")
                        nc.scalar.copy(out=qd_sb[:], in_=qd_ps[:])
                        k_sb = bsb.tile([128, 4, 128], BF16, tag="ksb")
                        nc.scalar.copy(out=k_sb[:], in_=k_ps[:])
                        prod = bsb.tile([128, 4, 128], BF16, tag="prod")
                        nc.vector.tensor_mul(prod[:], qd_sb[:], k_sb[:])
                        alpha = bsb.tile([128, 4, 8], F32, tag="alpha")
                        nc.vector.tensor_reduce(
                            out=alpha[:],
                            in_=prod[:].rearrange("e s (h c) -> e s h c", h=HEADS),
                            axis=mybir.AxisListType.X, op=mybir.AluOpType.add)
                        if has_bias:
                            nc.vector.tensor_add(alpha[:], alpha[:], be_ps)
                        payload = bsb.tile([128, 4, 136], BF16, tag="pay")
                        nc.scalar.activation(
                            out=payload[:, :, 128:136], in_=alpha[:],
                            func=mybir.ActivationFunctionType.Exp, scale=0.25)
                        nc.vector.tensor_mul(
                            payload[:, :, 0:128].rearrange("e s (h c) -> e s h c", h=HEADS),
                            v_ps[:].rearrange("e s (h c) -> e s h c", h=HEADS),
                            payload[:, :, 128:136].unsqueeze(3).to_broadcast([128, 4, 8, CPH]))
                        selT = bsb.tile([128, 4, 128], BF16, tag="selT")
                        nc.vector.tensor_tensor(
                            out=selT[:],
                            in0=dc_t.unsqueeze(2).to_broadcast([128, 4, 128]),
                            in1=m2_s[:].unsqueeze(1).to_broadcast([128, 4, 128]),
                            op=mybir.AluOpType.is_equal)
                        for s in range(4):
                            nc.tensor.matmul(
                                out=u_ps[:], lhsT=selT[:, s, :], rhs=payload[:, s, :],
                                start=(gi == 0 and s == 0),
                                stop=(gi == gw1 - 1 and s == 3),
                                skip_group_check=True)
                        xy_sb = bsb.tile([128, 4, 2], F32, tag="xysb")
                        nc.scalar.copy(out=xy_sb[:], in_=xy_ps)
                        nc.gpsimd.dma_start(out=xy_d[sl].rearrange("s e t -> e s t"), in_=xy_sb[:])
                    u_sb = bsb.tile([128, 136], F32, tag="usb")
                    nc.scalar.copy(out=u_sb[:], in_=u_ps[:])
                    nc.gpsimd.dma_start(out=u_d[w * WIN:(w + 1) * WIN, :], in_=u_sb[:])

            # ================= C: normalize + yA =================
            from concourse.masks import make_identity
            with tc.tile_pool(name="csb", bufs=3) as csb, \
                 tc.tile_pool(name="cps", bufs=2, space="PSUM") as cps:
                ident = csb.tile([128, 128], F32, tag="ident")
                make_identity(nc, ident[:])
                for w in range(n_win1):
                    u_sb = csb.tile([128, 136], F32, tag="cu")
                    nc.sync.dma_start(out=u_sb[:], in_=u_d[w * WIN:(w + 1) * WIN, :])
                    pc_t = csb.tile([128, 1], F32, tag="cpc")
                    nc.gpsimd.dma_start(out=pc_t[:], in_=pc_d[w])
                    dcr = csb.tile([128, 8], F32, tag="cdc")
                    nc.vector.tensor_sub(dcr[:], u_sb[:, 128:136],
                                         pc_t[:].to_broadcast([128, 8]))
                    den = csb.tile([128, 8], F32, tag="cden")
                    nc.vector.tensor_scalar_add(den[:], dcr[:], EPS)
                    rcp = csb.tile([128, 8], F32, tag="crcp")
                    nc.vector.reciprocal(out=rcp[:], in_=den[:])
                    agg = csb.tile([128, 128], F32, tag="cagg")
                    if has_bias:
                        nc.vector.tensor_mul(
                            agg[:].rearrange("n (h c) -> n h c", h=HEADS),
                            cv_s[:].rearrange("n (h c) -> n h c", h=HEADS),
                            dcr[:].unsqueeze(2).to_broadcast([128, 8, CPH]))
                        nc.vector.tensor_add(agg[:], agg[:], u_sb[:, 0:128])
                    else:
                        nc.vector.tensor_copy(out=agg[:], in_=u_sb[:, 0:128])
                    nc.vector.tensor_mul(
                        agg[:].rearrange("n (h c) -> n h c", h=HEADS),
                        agg[:].rearrange("n (h c) -> n h c", h=HEADS),
                        rcp[:].unsqueeze(2).to_broadcast([128, 8, CPH]))
                    aggT_ps = cps.tile([128, 128], F32, tag="caggT")
                    nc.tensor.transpose(out=aggT_ps[:], in_=agg[:], identity=ident[:])
                    aggT = csb.tile([128, 128], F32, tag="caggTs")
                    nc.scalar.copy(out=aggT[:], in_=aggT_ps[:])
                    ya_ps = cps.tile([128, 2], F32, tag="cya")
                    nc.tensor.matmul(out=ya_ps[:], lhsT=aggT[:], rhs=wc_s[:],
                                     start=True, stop=True, skip_group_check=True)
                    ya_sb = csb.tile([128, 2], F32, tag="cyasb")
                    nc.scalar.copy(out=ya_sb[:], in_=ya_ps[:])
                    nc.gpsimd.dma_start(out=ya_own_d[w * WIN:(w + 1) * WIN, :], in_=ya_sb[:])

                nc.gpsimd.collective_compute(
                    "AllGather", mybir.AluOpType.bypass,
                    replica_groups=[list(range(NCORES))],
                    ins=[ya_own_d[0:NPC, :]], outs=[ya_all_d[0:NCORES * NPC, :]])
                if n_nodes2 > NCORES * NPC:
                    zt = csb.tile([128, 2], F32, tag="czt")
                    nc.vector.memset(zt[:], 0.0)
                    pad = n_nodes2 - NCORES * NPC
                    nc.sync.dma_start(out=ya_all_d[NCORES * NPC:n_nodes2, :],
                                      in_=zt[0:pad, :])

            # ================= D: per-edge y selection =================
            # batches of 8 subtiles spanning `wspan` windows (wspan in {1,2})
            def d_pass(n_win, g_per_w, row_d, ytab_d, out_d, add_xy, pool_pfx):
                sub_per_w = g_per_w * 4
                wspan = max(1, 8 // sub_per_w)
                assert wspan == 1 or sub_per_w * wspan == 8
                n_batch_per_iter = max(1, sub_per_w // 8)
                with tc.tile_pool(name=pool_pfx + "sb", bufs=4) as dsb, \
                     tc.tile_pool(name=pool_pfx + "ps", bufs=2, space="PSUM") as dps:
                    for w0 in range(0, n_win, wspan):
                        nw = min(wspan, n_win - w0)
                        yw = dsb.tile([128, wspan, 2], F32, tag="dyw")
                        nc.sync.dma_start(
                            out=yw[:, 0:nw, :],
                            in_=ytab_d[w0 * WIN:(w0 + nw) * WIN, :]
                                .rearrange("(j n) t -> n j t", j=nw))
                        yw16 = dsb.tile([128, wspan, 2], FP16, tag="dyw16")
                        nc.scalar.copy(out=yw16[:, 0:nw, :], in_=yw[:, 0:nw, :])
                        for bi in range(n_batch_per_iter):
                            sub0 = w0 * sub_per_w + bi * 8
                            nsb = 8 if wspan == 1 else sub_per_w * nw
                            sl = slice(sub0, sub0 + nsb)
                            rows = dsb.tile([128, 8, 128], I8, tag="drow")
                            nc.sync.dma_start(
                                out=rows[:, 0:nsb, :],
                                in_=bass.AP(tensor=row_d[:].tensor, offset=sub0 * 128,
                                            ap=[[0, 128], [128, nsb], [1, 128]]))
                            sel = dsb.tile([128, 8, 128], FP16, tag="dsel")
                            nc.vector.tensor_tensor(
                                out=sel[:, 0:nsb, :],
                                in0=m1_s[:].unsqueeze(2).to_broadcast([128, nsb, 128]),
                                in1=rows[:, 0:nsb, :], op=mybir.AluOpType.is_equal)
                            smm = dps.tile([128, 8, 2], F32, tag="dsmm")
                            for s in range(nsb):
                                jw = s // sub_per_w if wspan > 1 else 0
                                nc.tensor.matmul(out=smm[:, s, :], lhsT=sel[:, s, :],
                                                 rhs=yw16[:, jw, :], start=True, stop=True,
                                                 skip_group_check=True)
                            o_sb = dsb.tile([128, 8, 2], F32, tag="dosb")
                            if add_xy:
                                xy_t = dsb.tile([128, 8, 2], F32, tag="dxy")
                                nc.gpsimd.dma_start(out=xy_t[:, 0:nsb, :],
                                                    in_=xy_d[sl].rearrange("s e t -> e s t"))
                                nc.vector.tensor_add(o_sb[:, 0:nsb, :], smm[:, 0:nsb, :],
                                                     xy_t[:, 0:nsb, :])
                            else:
                                nc.vector.tensor_copy(out=o_sb[:, 0:nsb, :], in_=smm[:, 0:nsb, :])
                            nc.gpsimd.dma_start(out=out_d[sl].rearrange("s e t -> e s t"),
                                                in_=o_sb[:, 0:nsb, :])

            d_pass(n_win1, gw1, dstrow_d, ya_own_d, out1_d, True, "d1")
            d_pass(n_win2, gw2, srcrow_d, ya_all_d, out2_d, False, "d2")

    nc.compile()
    return nc


_CACHE = {}


def _get_compiled(meta, has_bias):
    key = (meta["s_sub"], meta["s2"], meta["nsub1"], meta["nsub2"],
           meta["n_win1"], meta["n_win2"], has_bias)
    if key not in _CACHE:
        _CACHE[key] = build_program(meta, has_bias)
    return _CACHE[key]


def make_in_maps(per_core, consts):
    bfnp = mybir.dt.np(BF16)
    bf = lambda a: np.ascontiguousarray(a).astype(bfnp)
    m2 = np.tile(np.arange(128, dtype=np.float32)[None, :], (128, 1))
    m1 = np.arange(128, dtype=np.int8)[:, None]
    in_maps = []
    for pcd in per_core:
        in_maps.append({
            "pk": pcd["pk"], "dstrow": pcd["dstrow"],
            "srcrow": pcd["srcrow"], "pc": pcd["pc"],
            "wq": bf(consts["Wq2"]), "wk": bf(consts["Wk2"]),
            "wv": bf(consts["Wv2"]), "wpm": bf(consts["Wpm"]),
            "wsc": bf(consts["Wsc"]), "wc": np.ascontiguousarray(consts["Wc"]),
            "m2": bf(m2), "m1": m1,
            "cvrow": np.ascontiguousarray(consts["cv"][None, :]),
            "u1": bf(consts["U1"]), "u2": bf(consts["U2"]),
        })
    return in_maps


def assemble_output(res_list, per_core, consts, meta):
    out = np.zeros((E, NUM_CLASSES), np.float32)
    for c in range(NCORES):
        pcd = per_core[c]
        r = res_list[c]
        o1 = np.asarray(r["out1"], np.float32).reshape(-1, NUM_CLASSES)
        o2 = np.asarray(r["out2"], np.float32).reshape(-1, NUM_CLASSES)
        out[pcd["slots1"][pcd["valid1"]]] += o1[pcd["valid1"]]
        out[pcd["slots2"][pcd["valid2"]]] += o2[pcd["valid2"]]
    out += consts["out_const"][None, :]
    return out


def kernel(**inputs):
    from concourse.bass_utils import run_bass_kernel_spmd

    per_core, consts, meta = prep_host(inputs)
    nc = _get_compiled(meta, consts["has_bias"])
    in_maps = make_in_maps(per_core, consts)
    res = run_bass_kernel_spmd(nc, in_maps, list(range(NCORES)))
    return assemble_output(res.results, per_core, consts, meta)
